# revision 1
# baseline (speedup 1.0000x reference)
"""Trainium2 Bass kernel for nn_DART_Net (gnn_message_passing).

Math (per molecule b, node n):
  hi = mlp2_i(ai) * mask(ai)                 [B,N,128]
  h{j,k,l} = mlp2_t(a_t) * mask(a_t)        [B,N,M,128] -> sum over M
  atm = hi + sum_j + sum_k + sum_l
  out = (celu-chain(atm) @ W4.T + b4) * mask(ai)
with mlp2(x) = celu(celu(x@W1.T+b1)@W2.T+b2), celu alpha=0.1.

Device strategy (per core, pure data parallel over B: 8 molecules/core):
  - features on partitions, message rows on the free axis
  - alpha-shifted celu: with z' = z + b + alpha (bias+alpha delivered by the
    matmul itself: an extra ones-row for layer 1, a rank-1 ones matmul for
    later layers),
        e + alpha = max(z', min(alpha*e^{z'/alpha - 1}, alpha))
    so each celu costs ONE ACT op (Exp, const affine) plus ONE DVE
    scalar_tensor_tensor  (t min alpha) max z'.  The +alpha shift on every
    activation is folded into the next layer's host-precomputed bias.
  - sum over M: trunk layer-1 is linear => W1c @ sum_m e2 == sum_m W1c @ e2;
    accumulated into a phase-resident PSUM bank by matmuls with a repeated
    (step-0) output access pattern (PE fan-in).  Fallback: DVE reduce.
  - pad masks: the ai mask multiplies the output on device; exact-zero-sum
    message rows (probability ~0 for randn inputs) are fixed up via a
    host-computed per-node correction "corr" entering the same trunk matmul.
"""

import sys
import numpy as np
from contextlib import ExitStack

for _p in ("/opt/trn_rl_repo", "/root/.axon_site/_ro/trn_rl_repo"):
    if _p not in sys.path:
        sys.path.append(_p)

import ml_dtypes

BF16 = ml_dtypes.bfloat16

ALPHA = 0.1
INV_ALPHA = 1.0 / ALPHA
# exp bias so that Exp(z'/alpha + EXPB) == alpha * e^{(z'-alpha)/alpha}
EXPB = float(np.log(np.float32(ALPHA)) - 1.0)

B, N, M = 64, 128, 64
NCORES = 8
BL = B // NCORES          # molecules per core
LH, LO = 128, 128
C1, C2, C3 = 64, 32, 16

USE_FANIN = True          # PE fan-in m-reduce; False -> DVE tensor_reduce

_PROGRAM_CACHE = {}


# --------------------------------------------------------------------------
# device program
# --------------------------------------------------------------------------

def _build_program(nmol=BL, debug=False, use_fanin=USE_FANIN, reps=1):
    import concourse.bass as bass
    import concourse.tile as tile
    from concourse import bacc, mybir

    f32 = mybir.dt.float32
    bf16 = mybir.dt.bfloat16
    Alu = mybir.AluOpType
    Act = mybir.ActivationFunctionType

    nodes = nmol * N                  # nodes per core
    rmsg = nodes * M                  # message rows per tensor per core
    PH = min(512, nodes)              # nodes per trunk phase (1 PSUM bank)
    nphase = nodes // PH
    G = 1024                          # message columns per group
    rows_ph = PH * M                  # message rows per phase per tensor
    ngrp = rows_ph // G
    npg = G // M                      # nodes per group (16)

    nc = bacc.Bacc("TRN2", target_bir_lowering=False, debug=debug)

    # x tensors carry a 4th all-ones plane (delivers bias via the L1 matmul)
    x_dram = {t: nc.dram_tensor(f"x{t}", [4, rmsg], bf16, kind="ExternalInput")
              for t in "jkl"}
    xi_dram = nc.dram_tensor("xi", [4, nodes], bf16, kind="ExternalInput")
    corr_dram = nc.dram_tensor("corr", [LO, nodes], bf16, kind="ExternalInput")
    mi_dram = nc.dram_tensor("mi", [1, nodes], f32, kind="ExternalInput")

    w_specs = {}
    for t in "jkli":
        w_specs[f"w1{t}"] = ([4, LH], bf16)       # [W1.T ; b1+alpha]
        w_specs[f"w2{t}"] = ([LH, LO], bf16)      # W2.T
        w_specs[f"bb2{t}"] = ([1, LO], bf16)      # b2 + a - a*rowsum(W2)
    w_specs.update(wc1=([LO, C1], bf16), bbtr=([1, C1], f32),
                   wc2=([C1, C2], f32), bbc2=([1, C2], f32),
                   wc3=([C2, C3], f32), bbc3=([1, C3], f32),
                   wc4=([C3, 1], f32), bc4=([1, 1], f32))
    w_dram = {k: nc.dram_tensor(k, shp, dt, kind="ExternalInput")
              for k, (shp, dt) in w_specs.items()}
    out_dram = nc.dram_tensor("out", [1, nodes], f32, kind="ExternalOutput")

    with ExitStack() as ctx:
        tc = ctx.enter_context(tile.TileContext(nc))

        wpool = ctx.enter_context(tc.tile_pool(name="w", bufs=1))
        xpool = ctx.enter_context(tc.tile_pool(name="x", bufs=6))
        za_pool = ctx.enter_context(tc.tile_pool(name="za", bufs=2, space="PSUM"))
        zb_pool = ctx.enter_context(tc.tile_pool(name="zb", bufs=3, space="PSUM"))
        tr_pool = ctx.enter_context(tc.tile_pool(name="tr", bufs=1, space="PSUM"))
        t1_pool = ctx.enter_context(tc.tile_pool(name="t1", bufs=3))
        e1_pool = ctx.enter_context(tc.tile_pool(name="e1", bufs=3))
        t2_pool = ctx.enter_context(tc.tile_pool(name="t2", bufs=5))
        e2_pool = ctx.enter_context(tc.tile_pool(name="e2", bufs=3))
        small = ctx.enter_context(tc.tile_pool(name="small", bufs=2))

        wsb = {}
        for k, (shp, dt) in w_specs.items():
            wt = wpool.tile(shp, dt, tag=f"w_{k}")
            nc.sync.dma_start(wt[:], w_dram[k][:])
            wsb[k] = wt
        corr_sb = wpool.tile([LO, nodes], bf16, tag="corr")
        nc.sync.dma_start(corr_sb[:], corr_dram[:])
        mi_sb = wpool.tile([1, nodes], f32, tag="mi")
        nc.sync.dma_start(mi_sb[:], mi_dram[:])
        xi_sb = wpool.tile([4, nodes], bf16, tag="xi")
        nc.sync.dma_start(xi_sb[:], xi_dram[:])
        ones_sb = wpool.tile([1, 512], f32, tag="ones")
        nc.vector.memset(ones_sb[:], 1.0)
        ones_bf = wpool.tile([1, 512], bf16, tag="ones_bf")
        nc.vector.memset(ones_bf[:], 1.0)
        expb_sb = wpool.tile([128, 1], f32, tag="expb")
        nc.vector.memset(expb_sb[:], EXPB)

        if not use_fanin:
            atm_sb = wpool.tile([LO, nodes], f32, tag="atm")

        def celu_shift(zp, tt, out):
            """out = celu(zp - alpha) + alpha = max(zp, min(t, alpha));
            zp (PSUM) must already contain z + b + alpha."""
            P = zp.shape[0]
            nc.scalar.activation(tt, zp, Act.Exp, bias=expb_sb[:P, :],
                                 scale=INV_ALPHA)
            nc.vector.scalar_tensor_tensor(out, tt, ALPHA, zp,
                                           Alu.min, Alu.max)

        def bias_mm(zp, key, width):
            """accumulate per-partition bias into psum zp via rank-1 matmul"""
            ones = ones_bf if wsb[key].dtype == bf16 else ones_sb
            nc.tensor.matmul(zp, wsb[key][:], ones[:, :width],
                             start=False, stop=False, skip_group_check=True)

        rep_cm = tc.For_i(0, reps, 1) if reps > 1 else None
        if rep_cm is not None:
            ctx.enter_context(rep_cm)

        for p in range(nphase):
            nsl = slice(p * PH, (p + 1) * PH)      # node slice of this phase
            if use_fanin:
                trunk = tr_pool.tile([C1, PH], f32, tag="trunk")

            # ---- ai path (also initializes the trunk accumulation) ----
            zi = zb_pool.tile([LH, PH], f32, tag="zb")
            nc.tensor.matmul(zi[:], wsb["w1i"][:], xi_sb[:, nsl],
                             start=True, stop=True, skip_group_check=True)
            ti = t2_pool.tile([LH, PH], bf16, tag="t2")
            e1i = e1_pool.tile([LH, PH], bf16, tag="e1")
            celu_shift(zi[:], ti[:], e1i[:])

            zi2 = zb_pool.tile([LH, PH], f32, tag="zb")
            nc.tensor.matmul(zi2[:], wsb["w2i"][:], e1i[:], start=True,
                             stop=False, skip_group_check=True)
            bias_mm(zi2[:], "bb2i", PH)
            ti2 = t2_pool.tile([LH, PH], bf16, tag="t2")
            e2i = e2_pool.tile([LH, PH], bf16, tag="e2")
            celu_shift(zi2[:], ti2[:], e2i[:])

            if use_fanin:
                nc.tensor.matmul(trunk[:], wsb["wc1"][:], e2i[:],
                                 start=True, stop=False, skip_group_check=True)
                nc.tensor.matmul(trunk[:], wsb["wc1"][:], corr_sb[:, nsl],
                                 start=False, stop=False, skip_group_check=True)
                bias_mm(trunk[:], "bbtr", PH)
            else:
                nc.vector.tensor_copy(atm_sb[:, nsl], e2i[:])
                nc.vector.tensor_add(atm_sb[:, nsl], atm_sb[:, nsl],
                                     corr_sb[:, nsl])

            # ---- message streams j,k,l interleaved ----
            for g in range(ngrp):
                for t in "jkl":
                    off = p * rows_ph + g * G
                    xg = xpool.tile([4, G], bf16, tag="xg")
                    nc.sync.dma_start(xg[:], x_dram[t][:, off:off + G])

                    za = za_pool.tile([LH, G], f32, tag="za")
                    for h in range(2):
                        cs = slice(h * 512, (h + 1) * 512)
                        nc.tensor.matmul(za[:, cs], wsb[f"w1{t}"][:], xg[:, cs],
                                         start=True, stop=True)
                    t1 = t1_pool.tile([LH, G], bf16, tag="t1")
                    e1 = e1_pool.tile([LH, G], bf16, tag="e1")
                    celu_shift(za[:], t1[:], e1[:])

                    e2 = e2_pool.tile([LO, G], bf16, tag="e2")
                    if use_fanin:
                        # m-major layout: addr = m*npg + n
                        e2r = e2[:].rearrange("p (m n) -> p n m", m=M)
                    else:
                        e2r = e2[:].rearrange("p (n m) -> p n m", m=M)
                    for h in range(2):
                        cs = slice(h * 512, (h + 1) * 512)
                        zb = zb_pool.tile([LO, 512], f32, tag="zb")
                        nc.tensor.matmul(zb[:], wsb[f"w2{t}"][:], e1[:, cs],
                                         start=True, stop=False,
                                         skip_group_check=True)
                        bias_mm(zb[:], f"bb2{t}", 512)
                        t2 = t2_pool.tile([LO, 512], bf16, tag="t2")
                        if use_fanin:
                            ov = e2r[:, h * (npg // 2):(h + 1) * (npg // 2), :]
                        else:
                            ov = e2[:, cs]
                        celu_shift(zb[:], t2[:], ov)

                    if use_fanin:
                        # accumulate sum_m (wc1.T @ e2[:, (n,m)]) into
                        # trunk[:, g*npg:(g+1)*npg] via repeated out AP
                        tv = trunk[:, g * npg:(g + 1) * npg]
                        fan_ap = bass.AP(tensor=tv.tensor, offset=tv.offset,
                                         ap=[list(tv.ap[0]), [0, M // 2],
                                             list(tv.ap[-1])])
                        for mh in range(2):
                            cs = slice(mh * 512, (mh + 1) * 512)
                            nc.tensor.matmul(fan_ap, wsb["wc1"][:], e2[:, cs],
                                             start=False, stop=False,
                                             skip_group_check=True)
                    else:
                        red = small.tile([LO, npg], f32, tag="red")
                        nc.vector.tensor_reduce(red[:], e2r,
                                                axis=mybir.AxisListType.X,
                                                op=Alu.add)
                        asl = atm_sb[:, p * PH + g * npg:p * PH + (g + 1) * npg]
                        nc.vector.tensor_add(asl, asl, red[:])

            # ---- trunk chain for this phase ----
            if use_fanin:
                z1c = trunk[:]
            else:
                z1c_ps = tr_pool.tile([C1, PH], f32, tag="trunk")
                nc.tensor.matmul(z1c_ps[:], wsb["wc1"][:], atm_sb[:, nsl],
                                 start=True, stop=False, skip_group_check=True)
                bias_mm(z1c_ps[:], "bbtr", PH)
                z1c = z1c_ps[:]

            tc1 = t2_pool.tile([C1, PH], f32, tag="t2")
            ec1 = e1_pool.tile([C1, PH], f32, tag="e1")
            celu_shift(z1c, tc1[:], ec1[:])

            z2c = zb_pool.tile([C2, PH], f32, tag="zb")
            nc.tensor.matmul(z2c[:], wsb["wc2"][:], ec1[:], start=True,
                             stop=False, skip_group_check=True)
            bias_mm(z2c[:], "bbc2", PH)
            tc2 = t2_pool.tile([C2, PH], f32, tag="t2")
            ec2 = e1_pool.tile([C2, PH], f32, tag="e1")
            celu_shift(z2c[:], tc2[:], ec2[:])

            z3c = zb_pool.tile([C3, PH], f32, tag="zb")
            nc.tensor.matmul(z3c[:], wsb["wc3"][:], ec2[:], start=True,
                             stop=False, skip_group_check=True)
            bias_mm(z3c[:], "bbc3", PH)
            tc3 = t2_pool.tile([C3, PH], f32, tag="t2")
            ec3 = e1_pool.tile([C3, PH], f32, tag="e1")
            celu_shift(z3c[:], tc3[:], ec3[:])

            z4c = zb_pool.tile([1, PH], f32, tag="zb")
            nc.tensor.matmul(z4c[:], wsb["wc4"][:], ec3[:], start=True,
                             stop=True, skip_group_check=True)
            o = small.tile([1, PH], f32, tag="o")
            nc.scalar.activation(o[:], z4c[:], Act.Identity,
                                 bias=wsb["bc4"][:], scale=1.0)
            om = small.tile([1, PH], f32, tag="om")
            nc.vector.tensor_mul(om[:], o[:], mi_sb[:, nsl])
            nc.sync.dma_start(out_dram[:, nsl], om[:])

    nc.compile()
    return nc


# --------------------------------------------------------------------------
# host side
# --------------------------------------------------------------------------

def _celu_np(x):
    x = x.astype(np.float32)
    return (np.maximum(x, 0.0)
            + np.minimum(0.0, np.float32(ALPHA)
                         * np.expm1(x * np.float32(INV_ALPHA)))).astype(np.float32)


def _with_ones(flat):
    """[R,3] -> transposed-with-ones [4,R] contiguous."""
    r = flat.shape[0]
    out = np.empty((4, r), np.float32)
    out[:3] = flat.T
    out[3] = 1.0
    return out


def _prep_core(inputs, c, nmol=BL):
    s = slice(c * nmol, (c + 1) * nmol)
    nodes = nmol * N
    a = np.float32(ALPHA)
    d = {}
    for t, key in (("j", "aj"), ("k", "ak"), ("l", "al")):
        flat = np.ascontiguousarray(inputs[key][s], dtype=np.float32).reshape(-1, 3)
        d[f"x{t}"] = _with_ones(flat).astype(BF16)
    ai = np.ascontiguousarray(inputs["ai"][s], dtype=np.float32).reshape(-1, 3)
    d["xi"] = _with_ones(ai).astype(BF16)
    mi = ((ai[:, 0] + ai[:, 1]) + ai[:, 2]) != 0
    d["mi"] = mi.astype(np.float32)[None, :]

    corr = np.zeros((nodes, LO), np.float32)
    for key, wn in (("aj", "j"), ("ak", "k"), ("al", "l")):
        flat = np.ascontiguousarray(inputs[key][s], dtype=np.float32).reshape(-1, 3)
        ssum = (flat[:, 0] + flat[:, 1]) + flat[:, 2]
        idx = np.nonzero(ssum == 0)[0]
        if idx.size:
            W1 = inputs[f"W{wn}1"].astype(np.float32)
            b1 = inputs[f"b{wn}1"].astype(np.float32)
            W2 = inputs[f"W{wn}2"].astype(np.float32)
            b2 = inputs[f"b{wn}2"].astype(np.float32)
            h1 = _celu_np(flat[idx] @ W1.T + b1)
            h2 = _celu_np(h1 @ W2.T + b2)
            np.subtract.at(corr, idx // M, h2)
    d["corr"] = np.ascontiguousarray(corr.T).astype(BF16)

    def ct(x, dt=np.float32):
        return np.ascontiguousarray(x, dtype=np.float32).astype(dt)

    for t, wn in (("j", "j"), ("k", "k"), ("l", "l"), ("i", "i")):
        W1 = inputs[f"W{wn}1"].astype(np.float32)
        b1 = inputs[f"b{wn}1"].astype(np.float32)
        W2 = inputs[f"W{wn}2"].astype(np.float32)
        b2 = inputs[f"b{wn}2"].astype(np.float32)
        d[f"w1{t}"] = ct(np.vstack([W1.T, (b1 + a)[None, :]]), BF16)
        d[f"w2{t}"] = ct(W2.T, BF16)
        # input to L2 is e1+alpha -> subtract a*rowsum(W2); then +b2+alpha
        d[f"bb2{t}"] = ct(b2 + a - a * W2.sum(axis=1), BF16)[None, :]

    W1c = inputs["W1"].astype(np.float32); b1c = inputs["b1"].astype(np.float32)
    W2c = inputs["W2"].astype(np.float32); b2c = inputs["b2"].astype(np.float32)
    W3c = inputs["W3"].astype(np.float32); b3c = inputs["b3"].astype(np.float32)
    W4c = inputs["W4"].astype(np.float32); b4c = inputs["b4"].astype(np.float32)
    d["wc1"] = ct(W1c.T, BF16)
    # trunk input is the sum of 193 alpha-shifted activations (192 msgs + ai)
    d["bbtr"] = ct(b1c + a - (3 * M + 1) * a * W1c.sum(axis=1))[None, :]
    d["wc2"] = ct(W2c.T)
    d["bbc2"] = ct(b2c + a - a * W2c.sum(axis=1))[None, :]
    d["wc3"] = ct(W3c.T)
    d["bbc3"] = ct(b3c + a - a * W3c.sum(axis=1))[None, :]
    d["wc4"] = ct(W4c.T)
    d["bc4"] = ct(b4c - a * W4c.sum(axis=1))[:, None]
    return d


def _get_program(nmol=BL):
    key = (nmol, USE_FANIN)
    if key not in _PROGRAM_CACHE:
        _PROGRAM_CACHE[key] = _build_program(nmol=nmol, use_fanin=USE_FANIN)
    return _PROGRAM_CACHE[key]


def run(inputs, trace=False, **kwargs):
    """Returns (full_output [B,N,1] f32, BassKernelResults)."""
    from concourse.bass_utils import run_bass_kernel_spmd
    inputs = {k: np.asarray(v) for k, v in inputs.items()}
    nc = _get_program()
    in_maps = [_prep_core(inputs, c) for c in range(NCORES)]
    res = run_bass_kernel_spmd(nc, in_maps, core_ids=list(range(NCORES)),
                               trace=trace, **kwargs)
    outs = [res.results[c]["out"].reshape(BL, N, 1) for c in range(NCORES)]
    return np.concatenate(outs, axis=0).astype(np.float32), res


def kernel(**inputs):
    out, _ = run(inputs)
    return out

